# revision 9
# baseline (speedup 1.0000x reference)
"""Trainium2 Bass kernel for nn_BilinearHead (RMSNorm -> two 1x1 convs ->
bilinear scores at fixed index pairs + promo bias).

Math (per batch b):
    rms2[b]    = mean(x[b]**2) + eps
    f[b]       = from_w @ (x[b] * norm_weight) ;  t[b] = to_w @ (...)
    score[b,v] = <f[b,:,from_idx[v]], t[b,:,to_idx[v]]> / rms2[b]
                 + promo_bias[promo_idx[v]]
(valid because norm_weight == 1 and the conv biases are 0 for this problem's
input distribution; kernel() verifies and falls back to a host reference
otherwise).

Device algorithm (pure data parallel over batch: 8 cores x 128 batches).
Per core, with Gt_b = t_b^T f_b (the 64x64 bilinear matrix transposed):

  score[b, v] = Gt_b[to_idx[v], from_idx[v]] / rms2[b] + promo_row[v]

Host marshals x per core into [cp, b, par, hw] (c = 2*cp + par) so each DMA
reads 8 KB contiguous per partition (line-rate HBM instead of 512B runs).

Pipeline (batch groups of 16, software-pipelined one group deep on PE):
  1. DMA x group (f32) on alternating HWDGE queues (sync / scalar)
  2. GPSIMD cast f32 -> bf16 (xb) ; ACT Square(xb) -> x2 bf16 ; DVE
     segmented reduce -> z[cp, sigma(b)] bf16
  3. PE GEMM bf16 (FWL weight loads): both batch parities packed on psum
     partition halves via zero-padded stacked weights -> f, t
     ACT evicts f, DVE evicts t (bf16)
  4. PE 4-way-packed Gt matmuls (emitted one group behind the GEMMs so the
     PE never stalls on evictions): tile_position (64r, 64s), r = batch
     parity (psum bank), s = pair parity (partition half)
     -> gt3[(j, s), q, r, i] bf16  (ACT evicts bank A, DVE bank B)
  5. PE score matmuls, one (from-value i, 512-col chunk) segment at a time,
     columns sorted by from_idx: 2 row-group-packed MMs (s = 0 / 1) with
     rhs = duplicated one-hot(to_idx) -> psum rows sigma(b) = 64s + 2q + r
  6. DVE fused per chunk: out = score * invrms2 + promo_sorted -> DMA out
  7. Host: un-permute rows (sigma) and columns (from_idx sort order).
"""

import sys

sys.path.insert(0, "/opt/trn_rl_repo")

import numpy as np

import concourse.bass as bass
import concourse.tile as tile
from concourse import mybir
from concourse.bacc import Bacc
from concourse.bass_utils import run_bass_kernel_spmd

# Problem shape (hardcoded per contest contract)
B_TOT, C, HW, D, V = 1024, 256, 8 * 8, 64, 1968
N_CORES = 8
B = B_TOT // N_CORES  # 128 batches per core
CP = C // 2  # 128 channel pairs (partition dim for GEMM)
NGROUPS = 8
GB = B // NGROUPS  # 16 batches per group
PAIRS_PER_GROUP = GB // 2
QUADS_PER_GROUP = GB // 4
NQUADS = B // 4  # 32
EPS = 1e-6
F32 = mybir.dt.float32
BF16 = mybir.dt.bfloat16


def _sigma():
    """Partition index of batch b in the score psum: 64*s + 2*q + r where
    b = 4*q + 2*s + r."""
    b = np.arange(B)
    return (64 * ((b >> 1) & 1) + 2 * (b >> 2) + (b & 1)).astype(np.int64)


def build_kernel(seg_plan):
    """seg_plan: list of (i, col0, ncols) score-matmul segments, where i is
    the from_idx value, col0 the starting column in from_idx-sorted order,
    and the segment does not cross a 512 psum-bank boundary."""
    nc = Bacc()

    # x pre-marshalled on host to [cp, b, par, hw]
    xs = nc.dram_tensor("xs", [CP, B, 2, HW], F32, kind="ExternalInput")
    w_f_lo = nc.dram_tensor("w_f_lo", [2, CP, 128], BF16, kind="ExternalInput")
    w_f_hi = nc.dram_tensor("w_f_hi", [2, CP, 128], BF16, kind="ExternalInput")
    w_t_lo = nc.dram_tensor("w_t_lo", [2, CP, 128], BF16, kind="ExternalInput")
    w_t_hi = nc.dram_tensor("w_t_hi", [2, CP, 128], BF16, kind="ExternalInput")
    ident = nc.dram_tensor("ident", [128, 128], F32, kind="ExternalInput")
    s_onehot2 = nc.dram_tensor("s_onehot2", [128, V], BF16, kind="ExternalInput")
    promo_rep = nc.dram_tensor("promo_rep", [128, V], F32, kind="ExternalInput")
    out = nc.dram_tensor("out", [B, V], F32, kind="ExternalOutput")

    with tile.TileContext(nc) as tc:
        with (
            tc.tile_pool(name="const", bufs=1) as const,
            tc.tile_pool(name="xin", bufs=NGROUPS) as xin,
            tc.tile_pool(name="xbp", bufs=3) as xbp,
            tc.tile_pool(name="x2p", bufs=2) as x2p,
            tc.tile_pool(name="psmm", bufs=2, space="PSUM") as psmm,
            tc.tile_pool(name="psgt", bufs=1, space="PSUM") as psgt,
            tc.tile_pool(name="pssc", bufs=2, space="PSUM") as pssc,
        ):
            # ---- constants (SWDGE queue, away from the x stream) ----
            wf_lo = const.tile([CP, 2, 128], BF16)
            wf_hi = const.tile([CP, 2, 128], BF16)
            wt_lo = const.tile([CP, 2, 128], BF16)
            wt_hi = const.tile([CP, 2, 128], BF16)
            for t_sb, t_dr in (
                (wf_lo, w_f_lo),
                (wf_hi, w_f_hi),
                (wt_lo, w_t_lo),
                (wt_hi, w_t_hi),
            ):
                nc.gpsimd.dma_start(
                    out=t_sb, in_=t_dr[:, :, :].rearrange("par cp m -> cp par m")
                )
            ident_sb = const.tile([128, 128], F32)
            nc.gpsimd.dma_start(out=ident_sb, in_=ident[:, :])
            onehot_sb = const.tile([128, V], BF16)
            promo_sb = const.tile([128, V], F32)

            # ---- persistent working tiles ----
            f_sb = const.tile([128, B // 2, HW], BF16)  # [(d, r), pair, i]
            t_sb = const.tile([128, B // 2, HW], BF16)
            gt3 = const.tile([128, NQUADS, 2, D], BF16)  # [(j, s), q, r, i]
            z = const.tile([128, B], BF16)  # [cp, sigma(b)] x^2 partial sums
            z2 = const.tile([128, B], F32)
            final_sb = const.tile([128, V], F32)
            inv_sb = const.tile([128, 1], F32)

            # sigma-ordered view of z: col = 64*s + 2*q + r
            z_v = z[:, :].rearrange("p (s g q r) -> p g q s r", s=2, g=NGROUPS, q=4, r=2)

            n_chunks = (V + 511) // 512

            # PE warmup: dummy matmuls so the HAM clock-gate opens (K=8/8)
            # before the first real GEMM; results are never read.
            warm_ps = pssc.tile([128, 512], F32, tag="sc")
            wrhs = wf_lo[:, :, :].rearrange("p a b -> p (a b)")
            for _wu in range(20):
                nc.tensor.matmul(
                    out=warm_ps[:, 0:256],
                    lhsT=wf_lo[:, 0, :],
                    rhs=wrhs,
                    start=True,
                    stop=True,
                )

            # ---- software-pipelined loop over batch groups ----
            # iteration g emits: DMA/cast/GEMM/evicts for group g, then the
            # Gt matmuls + evictions for group g-1 (so the PE always has the
            # next GEMM queued while evictions catch up).
            def emit_gt(g):
                psA = psgt.tile([128, 2 * QUADS_PER_GROUP, D], F32, tag="gA")
                psB = psgt.tile([128, 2 * QUADS_PER_GROUP, D], F32, tag="gB")
                for q4 in range(QUADS_PER_GROUP):
                    for s in range(2):
                        k = g * PAIRS_PER_GROUP + 2 * q4 + s
                        for r, ps_gt in ((0, psA), (1, psB)):
                            nc.tensor.matmul(
                                out=ps_gt[64 * s : 64 * s + 64, q4, :],
                                lhsT=t_sb[64 * r : 64 * r + 64, k, :],
                                rhs=f_sb[64 * r : 64 * r + 64, k, :],
                                start=True,
                                stop=True,
                                tile_position=(64 * r, 64 * s),
                            )
                return psA, psB

            def emit_gt_evict(psA, psB, g):
                q0 = g * QUADS_PER_GROUP
                q1 = q0 + QUADS_PER_GROUP
                nc.scalar.copy(
                    out=gt3[:, q0:q1, 0, :], in_=psA[:, 0:QUADS_PER_GROUP, :]
                )
                nc.vector.tensor_copy(
                    out=gt3[:, q0:q1, 1, :], in_=psB[:, 0:QUADS_PER_GROUP, :]
                )

            pend = None  # (psA, psB, g) awaiting Gt eviction
            for g in range(NGROUPS):
                b0 = g * GB
                xt = xin.tile([CP, GB, 2, HW], F32)
                ring = nc.sync if g % 2 == 0 else nc.scalar
                ring.dma_start(out=xt, in_=xs[:, b0 : b0 + GB, :, :])

                # GPSIMD cast f32 -> bf16 for the GEMM and the square
                xb = xbp.tile([128, GB, 2 * HW], BF16)
                nc.gpsimd.tensor_copy(
                    out=xb[:, :, :],
                    in_=xt[:, :, :, :].rearrange("p b par hw -> p b (par hw)"),
                )

                # GEMMs (bf16, FWL): psum rows 0-63 = even-batch d,
                # rows 64-127 = odd-batch d
                xv = xb[:, :, :].rearrange(
                    "p (pr two) (par hw) -> p pr two par hw", two=2, par=2
                )
                pf = psmm.tile([128, PAIRS_PER_GROUP, HW], F32, tag="pf")
                pt = psmm.tile([128, PAIRS_PER_GROUP, HW], F32, tag="pt")
                for ps, wlo, whi in ((pf, wf_lo, wf_hi), (pt, wt_lo, wt_hi)):
                    for mi in range(4):
                        half, par0 = mi // 2, mi % 2
                        w_sb = whi if half else wlo
                        nc.tensor.matmul(
                            out=ps[:, :, :],
                            lhsT=w_sb[:, par0, :],
                            rhs=xv[:, :, half, par0, :],
                            start=(mi == 0),
                            stop=(mi == 3),
                        )
                p0 = g * PAIRS_PER_GROUP
                p1 = p0 + PAIRS_PER_GROUP
                nc.scalar.copy(out=f_sb[:, p0:p1, :], in_=pf[:, :, :])
                nc.vector.tensor_copy(out=t_sb[:, p0:p1, :], in_=pt[:, :, :])

                # previous group's Gt eviction, then this group's Gt matmuls
                if pend is not None:
                    psA, psB, gp = pend
                    emit_gt_evict(psA, psB, gp)
                pend_gt = emit_gt(g)

                # x^2 (ACT, from bf16) and segmented reduce (DVE) -> z
                x2t = x2p.tile([128, GB, 2 * HW], BF16)
                nc.scalar.activation(
                    out=x2t[:, :, :],
                    in_=xb[:, :, :],
                    func=mybir.ActivationFunctionType.Square,
                )
                red_in = x2t[:, :, :].rearrange(
                    "p (q s r) hw -> p q s r hw", q=4, s=2, r=2
                )
                with nc.allow_low_precision(
                    reason="bf16 partial x^2 sums: |err| ~0.02% of rms2"
                ):
                    nc.vector.tensor_reduce(
                        out=z_v[:, g, :, :, :],
                        in_=red_in,
                        axis=mybir.AxisListType.X,
                        op=mybir.AluOpType.add,
                    )
                pend = (*pend_gt, g)

            psA, psB, gp = pend
            emit_gt_evict(psA, psB, gp)

            # ---- 1/rms2 per batch (sigma order) ----
            nc.vector.tensor_copy(out=z2[:, :], in_=z[:, :])
            zt_ps = psmm.tile([128, PAIRS_PER_GROUP, HW], F32, tag="pf")
            zt_flat = zt_ps[:, :, :].rearrange("p a b -> p (a b)")
            nc.tensor.transpose(out=zt_flat[:, 0:128], in_=z2[:, :], identity=ident_sb[:, :])
            nc.vector.tensor_reduce(
                out=inv_sb[:, :],
                in_=zt_flat[:, 0:128],
                axis=mybir.AxisListType.X,
                op=mybir.AluOpType.add,
            )
            nc.vector.tensor_scalar(
                out=inv_sb[:, :],
                in0=inv_sb[:, :],
                scalar1=1.0 / (C * HW),
                scalar2=EPS,
                op0=mybir.AluOpType.mult,
                op1=mybir.AluOpType.add,
            )
            nc.vector.reciprocal(out=inv_sb[:, :], in_=inv_sb[:, :])

            nc.gpsimd.dma_start(out=onehot_sb, in_=s_onehot2[:, :])
            nc.gpsimd.dma_start(out=promo_sb, in_=promo_rep[:, :])

            # ---- score matmuls (columns in from_idx-sorted order) ----
            # Per segment, 2 row-group-packed MMs (s = 0 / 1): contraction
            # over j in gt3 rows 64s..64s+63, out partitions 64s + (2q + r).
            by_chunk = [[] for _ in range(n_chunks)]
            for i, col0, ncols in seg_plan:
                by_chunk[col0 // 512].append((i, col0, ncols))
            for q in range(n_chunks):
                sc_chunk = pssc.tile([128, 512], F32, tag="sc")
                for i, col0, ncols in by_chunk[q]:
                    c0 = col0 % 512
                    for s in range(2):
                        nc.tensor.matmul(
                            out=sc_chunk[64 * s : 64 * s + 64, c0 : c0 + ncols],
                            lhsT=gt3[64 * s : 64 * s + 64, :, :, i],
                            rhs=onehot_sb[64 * s : 64 * s + 64, col0 : col0 + ncols],
                            start=True,
                            stop=True,
                            tile_position=(64 * s, 64 * s),
                        )
                # fused: out = score * invrms2 + promo (sorted order)
                n = min(512, V - q * 512)
                cols = slice(q * 512, q * 512 + n)
                nc.vector.scalar_tensor_tensor(
                    out=final_sb[:, cols],
                    in0=sc_chunk[:, 0:n],
                    scalar=inv_sb[:, 0:1],
                    in1=promo_sb[:, cols],
                    op0=mybir.AluOpType.mult,
                    op1=mybir.AluOpType.add,
                )
                nc.sync.dma_start(out=out[:, cols], in_=final_sb[:, cols])

    nc.compile()
    return nc


_NC_CACHE = {}


def _plan_from_indices(from_idx, to_idx):
    from_idx = np.asarray(from_idx, np.int64)
    to_idx = np.asarray(to_idx, np.int64)
    order = np.argsort(from_idx, kind="stable")
    fi_sorted = from_idx[order]
    seg_plan = []
    col = 0
    for i in range(HW):
        n = int(np.count_nonzero(fi_sorted == i))
        while n > 0:
            m = min(n, 512 - col % 512)
            seg_plan.append((i, col, m))
            col += m
            n -= m
    assert col == V
    # duplicated one-hot: rows j and j+64 both = 1[to_idx[order[s]] == j]
    onehot2 = np.zeros((128, V), np.float32)
    onehot2[to_idx[order], np.arange(V)] = 1.0
    onehot2[to_idx[order] + 64, np.arange(V)] = 1.0
    return tuple(seg_plan), order, onehot2


def _host_inputs(from_w, to_w):
    def stack_w(wmat):
        wt = np.ascontiguousarray(wmat.T).reshape(CP, 2, D)  # [cp, par, d]
        lo = np.zeros((2, CP, 128), np.float32)
        hi = np.zeros((2, CP, 128), np.float32)
        lo[:, :, 0:D] = wt.transpose(1, 0, 2)
        hi[:, :, D:128] = wt.transpose(1, 0, 2)
        bf = mybir.dt.np(BF16)
        return lo.astype(bf), hi.astype(bf)

    wf_lo, wf_hi = stack_w(np.asarray(from_w, np.float32))
    wt_lo, wt_hi = stack_w(np.asarray(to_w, np.float32))
    return wf_lo, wf_hi, wt_lo, wt_hi


def _prepare(x, from_w, to_w, promo_bias, from_idx, to_idx, promo_idx):
    """Build (nc, in_maps, postprocess) for the device run."""
    seg_plan, order, onehot2 = _plan_from_indices(from_idx, to_idx)
    if seg_plan not in _NC_CACHE:
        _NC_CACHE[seg_plan] = build_kernel(seg_plan)
    nc = _NC_CACHE[seg_plan]

    wf_lo, wf_hi, wt_lo, wt_hi = _host_inputs(from_w, to_w)
    promo_sorted = np.asarray(promo_bias, np.float32)[
        np.asarray(promo_idx, np.int64)
    ][order]
    promo_rep = np.ascontiguousarray(
        np.broadcast_to(promo_sorted[None, :], (128, V)), np.float32
    )
    # marshal x per core into [cp, b, par, hw] (c = 2*cp + par)
    xr = np.asarray(x, np.float32).reshape(N_CORES, B, CP, 2, HW)
    xr = np.ascontiguousarray(xr.transpose(0, 2, 1, 3, 4))
    shared = {
        "w_f_lo": wf_lo,
        "w_f_hi": wf_hi,
        "w_t_lo": wt_lo,
        "w_t_hi": wt_hi,
        "ident": np.eye(128, dtype=np.float32),
        "s_onehot2": onehot2.astype(mybir.dt.np(BF16)),
        "promo_rep": promo_rep,
    }
    in_maps = [dict(shared, xs=xr[c]) for c in range(N_CORES)]

    sigma = _sigma()

    def post(res):
        full = np.empty((B_TOT, V), np.float32)
        for c in range(N_CORES):
            raw = res.results[c]["out"]  # rows sigma(b), cols sorted order
            logical = raw[sigma]  # rows b, cols sorted order
            full[c * B : (c + 1) * B][:, order] = logical
        return full

    return nc, in_maps, post


def kernel(
    x,
    norm_weight,
    from_w,
    from_b,
    to_w,
    to_b,
    promo_bias,
    from_idx,
    to_idx,
    promo_idx,
):
    x = np.asarray(x, np.float32)
    norm_weight = np.asarray(norm_weight, np.float32)
    from_b = np.asarray(from_b, np.float32)
    to_b = np.asarray(to_b, np.float32)

    if (
        np.any(from_b != 0.0)
        or np.any(to_b != 0.0)
        or not np.allclose(norm_weight, 1.0)
    ):
        # General-correctness fallback; never hit for this problem's input
        # distribution (norm_weight is ones, conv biases are zeros).
        return _host_reference(
            x, norm_weight, from_w, from_b, to_w, to_b, promo_bias,
            from_idx, to_idx, promo_idx,
        )

    nc, in_maps, post = _prepare(
        x, from_w, to_w, promo_bias, from_idx, to_idx, promo_idx
    )
    res = run_bass_kernel_spmd(nc, in_maps, core_ids=list(range(N_CORES)))
    return post(res)


def _host_reference(
    x, norm_weight, from_w, from_b, to_w, to_b, promo_bias, from_idx, to_idx, promo_idx
):
    b, c, w, h = x.shape
    rms = np.sqrt(np.mean(x * x, axis=(1, 2, 3), keepdims=True) + EPS)
    xn = (x / rms) * norm_weight[None]
    f = (
        np.einsum("bchw,dc->bdhw", xn, from_w) + from_b[None, :, None, None]
    ).reshape(b, -1, w * h)
    t = (
        np.einsum("bchw,dc->bdhw", xn, to_w) + to_b[None, :, None, None]
    ).reshape(b, -1, w * h)
    score = np.einsum("bdv,bdv->bv", f[:, :, from_idx], t[:, :, to_idx])
    return (score + promo_bias[promo_idx][None, :]).astype(np.float32)


# revision 10
# speedup vs baseline: 1.5000x; 1.5000x over previous
"""Trainium2 Bass kernel for nn_BilinearHead (RMSNorm -> two 1x1 convs ->
bilinear scores at fixed index pairs + promo bias).

Math (per batch b):
    rms2[b]    = mean(x[b]**2) + eps
    f[b]       = from_w @ (x[b] * norm_weight) ;  t[b] = to_w @ (...)
    score[b,v] = <f[b,:,from_idx[v]], t[b,:,to_idx[v]]> / rms2[b]
                 + promo_bias[promo_idx[v]]
(valid because norm_weight == 1 and the conv biases are 0 for this problem's
input distribution; kernel() verifies and falls back to a host reference
otherwise).

Device algorithm (pure data parallel over batch: 8 cores x 128 batches).
Per core, with Gt_b = t_b^T f_b (the 64x64 bilinear matrix transposed):

  score[b, v] = Gt_b[to_idx[v], from_idx[v]] / rms2[b] + promo_row[v]

Host marshals x per core into [cp, b, par, hw] (c = 2*cp + par) so each DMA
reads 8 KB contiguous per partition (line-rate HBM instead of 512B runs).

Pipeline (batch groups of 16, software-pipelined one group deep on PE):
  1. DMA x group (f32) on alternating HWDGE queues (sync / scalar)
  2. GPSIMD cast f32 -> bf16 (xb) ; ACT Square(xb) -> x2 bf16 ; DVE
     segmented reduce -> z[cp, sigma(b)] bf16
  3. PE GEMM bf16 (FWL weight loads): both batch parities packed on psum
     partition halves via zero-padded stacked weights -> f, t
     ACT evicts f, DVE evicts t (bf16)
  4. PE 4-way-packed Gt matmuls (emitted one group behind the GEMMs so the
     PE never stalls on evictions): tile_position (64r, 64s), r = batch
     parity (psum bank), s = pair parity (partition half)
     -> gt3[(j, s), q, r, i] bf16  (ACT evicts bank A, DVE bank B)
  5. PE score matmuls, one (from-value i, 512-col chunk) segment at a time,
     columns sorted by from_idx: 2 row-group-packed MMs (s = 0 / 1) with
     rhs = duplicated one-hot(to_idx) -> psum rows sigma(b) = 64s + 2q + r
  6. DVE fused per chunk: out = score * invrms2 + promo_sorted -> DMA out
  7. Host: un-permute rows (sigma) and columns (from_idx sort order).
"""

import sys

sys.path.insert(0, "/opt/trn_rl_repo")

import numpy as np

import concourse.bass as bass
import concourse.tile as tile
from concourse import mybir
from concourse.bacc import Bacc
from concourse.bass_utils import run_bass_kernel_spmd

# Problem shape (hardcoded per contest contract)
B_TOT, C, HW, D, V = 1024, 256, 8 * 8, 64, 1968
N_CORES = 8
B = B_TOT // N_CORES  # 128 batches per core
CP = C // 2  # 128 channel pairs (partition dim for GEMM)
NGROUPS = 8
GB = B // NGROUPS  # 16 batches per group
PAIRS_PER_GROUP = GB // 2
QUADS_PER_GROUP = GB // 4
NQUADS = B // 4  # 32
EPS = 1e-6
F32 = mybir.dt.float32
BF16 = mybir.dt.bfloat16


def _sigma():
    """Partition index of batch b in the score psum: 64*s + 2*q + r where
    b = 4*q + 2*s + r."""
    b = np.arange(B)
    return (64 * ((b >> 1) & 1) + 2 * (b >> 2) + (b & 1)).astype(np.int64)


def build_kernel(seg_plan):
    """seg_plan: list of (i, col0, ncols) score-matmul segments, where i is
    the from_idx value, col0 the starting column in from_idx-sorted order,
    and the segment does not cross a 512 psum-bank boundary."""
    nc = Bacc()

    # x pre-marshalled on host to [cp, b, par, hw]
    xs = nc.dram_tensor("xs", [CP, B, 2, HW], F32, kind="ExternalInput")
    w_f_lo = nc.dram_tensor("w_f_lo", [2, CP, 128], BF16, kind="ExternalInput")
    w_f_hi = nc.dram_tensor("w_f_hi", [2, CP, 128], BF16, kind="ExternalInput")
    w_t_lo = nc.dram_tensor("w_t_lo", [2, CP, 128], BF16, kind="ExternalInput")
    w_t_hi = nc.dram_tensor("w_t_hi", [2, CP, 128], BF16, kind="ExternalInput")
    ident = nc.dram_tensor("ident", [128, 128], F32, kind="ExternalInput")
    s_onehot2 = nc.dram_tensor("s_onehot2", [128, V], BF16, kind="ExternalInput")
    promo_rep = nc.dram_tensor("promo_rep", [128, V], F32, kind="ExternalInput")
    out = nc.dram_tensor("out", [B, V], F32, kind="ExternalOutput")

    with tile.TileContext(nc) as tc:
        with (
            tc.tile_pool(name="const", bufs=1) as const,
            tc.tile_pool(name="xbp", bufs=NGROUPS) as xbp,
            tc.tile_pool(name="x2p", bufs=2) as x2p,
            tc.tile_pool(name="psmm", bufs=2, space="PSUM") as psmm,
            tc.tile_pool(name="psgt", bufs=1, space="PSUM") as psgt,
            tc.tile_pool(name="pssc", bufs=2, space="PSUM") as pssc,
        ):
            # ---- constants (SWDGE queue, away from the x stream) ----
            wf_lo = const.tile([CP, 2, 128], BF16)
            wf_hi = const.tile([CP, 2, 128], BF16)
            wt_lo = const.tile([CP, 2, 128], BF16)
            wt_hi = const.tile([CP, 2, 128], BF16)
            for t_sb, t_dr in (
                (wf_lo, w_f_lo),
                (wf_hi, w_f_hi),
                (wt_lo, w_t_lo),
                (wt_hi, w_t_hi),
            ):
                nc.gpsimd.dma_start(
                    out=t_sb, in_=t_dr[:, :, :].rearrange("par cp m -> cp par m")
                )
            ident_sb = const.tile([128, 128], F32)
            nc.gpsimd.dma_start(out=ident_sb, in_=ident[:, :])
            onehot_sb = const.tile([128, V], BF16)
            promo_sb = const.tile([128, V], F32)

            # ---- persistent working tiles ----
            f_sb = const.tile([128, B // 2, HW], BF16)  # [(d, r), pair, i]
            t_sb = const.tile([128, B // 2, HW], BF16)
            gt3 = const.tile([128, NQUADS, 2, D], BF16)  # [(j, s), q, r, i]
            z = const.tile([128, B], BF16)  # [cp, sigma(b)] x^2 partial sums
            z2 = const.tile([128, B], F32)
            final_sb = const.tile([128, V], F32)
            inv_sb = const.tile([128, 1], F32)

            # sigma-ordered view of z: col = 64*s + 2*q + r
            z_v = z[:, :].rearrange("p (s g q r) -> p g q s r", s=2, g=NGROUPS, q=4, r=2)

            n_chunks = (V + 511) // 512

            # PE warmup: dummy matmuls so the HAM clock-gate opens (K=8/8)
            # before the first real GEMM; results are never read.
            warm_ps = pssc.tile([128, 512], F32, tag="sc")
            wrhs = wf_lo[:, :, :].rearrange("p a b -> p (a b)")
            for _wu in range(20):
                nc.tensor.matmul(
                    out=warm_ps[:, 0:256],
                    lhsT=wf_lo[:, 0, :],
                    rhs=wrhs,
                    start=True,
                    stop=True,
                )

            # ---- software-pipelined loop over batch groups ----
            # iteration g emits: DMA/cast/GEMM/evicts for group g, then the
            # Gt matmuls + evictions for group g-1 (so the PE always has the
            # next GEMM queued while evictions catch up).
            def emit_gt(g):
                psA = psgt.tile([128, 2 * QUADS_PER_GROUP, D], F32, tag="gA")
                psB = psgt.tile([128, 2 * QUADS_PER_GROUP, D], F32, tag="gB")
                for q4 in range(QUADS_PER_GROUP):
                    for s in range(2):
                        k = g * PAIRS_PER_GROUP + 2 * q4 + s
                        for r, ps_gt in ((0, psA), (1, psB)):
                            nc.tensor.matmul(
                                out=ps_gt[64 * s : 64 * s + 64, q4, :],
                                lhsT=t_sb[64 * r : 64 * r + 64, k, :],
                                rhs=f_sb[64 * r : 64 * r + 64, k, :],
                                start=True,
                                stop=True,
                                tile_position=(64 * r, 64 * s),
                            )
                return psA, psB

            def emit_gt_evict(psA, psB, g):
                q0 = g * QUADS_PER_GROUP
                q1 = q0 + QUADS_PER_GROUP
                nc.scalar.copy(
                    out=gt3[:, q0:q1, 0, :], in_=psA[:, 0:QUADS_PER_GROUP, :]
                )
                nc.scalar.copy(
                    out=gt3[:, q0:q1, 1, :], in_=psB[:, 0:QUADS_PER_GROUP, :]
                )

            pend = None  # (psA, psB, g) awaiting Gt eviction
            for g in range(NGROUPS):
                b0 = g * GB
                # SWDGE DMA with f32 -> bf16 cast in flight (contiguous 8KB
                # HBM runs per partition thanks to the host marshalling)
                xb = xbp.tile([128, GB, 2 * HW], BF16)
                nc.gpsimd.dma_start(
                    out=xb[:, :, :],
                    in_=xs[:, b0 : b0 + GB, :, :].rearrange(
                        "p b par hw -> p b (par hw)"
                    ),
                )

                # GEMMs (bf16, FWL): psum rows 0-63 = even-batch d,
                # rows 64-127 = odd-batch d
                xv = xb[:, :, :].rearrange(
                    "p (pr two) (par hw) -> p pr two par hw", two=2, par=2
                )
                pf = psmm.tile([128, PAIRS_PER_GROUP, HW], F32, tag="pf")
                pt = psmm.tile([128, PAIRS_PER_GROUP, HW], F32, tag="pt")
                for ps, wlo, whi in ((pf, wf_lo, wf_hi), (pt, wt_lo, wt_hi)):
                    for mi in range(4):
                        half, par0 = mi // 2, mi % 2
                        w_sb = whi if half else wlo
                        nc.tensor.matmul(
                            out=ps[:, :, :],
                            lhsT=w_sb[:, par0, :],
                            rhs=xv[:, :, half, par0, :],
                            start=(mi == 0),
                            stop=(mi == 3),
                        )
                p0 = g * PAIRS_PER_GROUP
                p1 = p0 + PAIRS_PER_GROUP
                nc.scalar.copy(out=f_sb[:, p0:p1, :], in_=pf[:, :, :])
                nc.scalar.copy(out=t_sb[:, p0:p1, :], in_=pt[:, :, :])

                # previous group's Gt eviction, then this group's Gt matmuls
                if pend is not None:
                    psA, psB, gp = pend
                    emit_gt_evict(psA, psB, gp)
                pend_gt = emit_gt(g)

                # x^2 (DVE tensor_tensor, bf16 2x) and segmented reduce -> z
                x2t = x2p.tile([128, GB, 2 * HW], BF16)
                nc.vector.tensor_tensor(
                    out=x2t[:, :, :],
                    in0=xb[:, :, :],
                    in1=xb[:, :, :],
                    op=mybir.AluOpType.mult,
                )
                red_in = x2t[:, :, :].rearrange(
                    "p (q s r) hw -> p q s r hw", q=4, s=2, r=2
                )
                with nc.allow_low_precision(
                    reason="bf16 partial x^2 sums: |err| ~0.02% of rms2"
                ):
                    nc.vector.tensor_reduce(
                        out=z_v[:, g, :, :, :],
                        in_=red_in,
                        axis=mybir.AxisListType.X,
                        op=mybir.AluOpType.add,
                    )
                pend = (*pend_gt, g)

            psA, psB, gp = pend
            emit_gt_evict(psA, psB, gp)

            # ---- 1/rms2 per batch (sigma order) ----
            nc.vector.tensor_copy(out=z2[:, :], in_=z[:, :])
            zt_ps = psmm.tile([128, PAIRS_PER_GROUP, HW], F32, tag="pf")
            zt_flat = zt_ps[:, :, :].rearrange("p a b -> p (a b)")
            nc.tensor.transpose(out=zt_flat[:, 0:128], in_=z2[:, :], identity=ident_sb[:, :])
            nc.vector.tensor_reduce(
                out=inv_sb[:, :],
                in_=zt_flat[:, 0:128],
                axis=mybir.AxisListType.X,
                op=mybir.AluOpType.add,
            )
            nc.vector.tensor_scalar(
                out=inv_sb[:, :],
                in0=inv_sb[:, :],
                scalar1=1.0 / (C * HW),
                scalar2=EPS,
                op0=mybir.AluOpType.mult,
                op1=mybir.AluOpType.add,
            )
            nc.vector.reciprocal(out=inv_sb[:, :], in_=inv_sb[:, :])

            nc.gpsimd.dma_start(out=onehot_sb, in_=s_onehot2[:, :])
            nc.gpsimd.dma_start(out=promo_sb, in_=promo_rep[:, :])

            # ---- score matmuls (columns in from_idx-sorted order) ----
            # Per segment, 2 row-group-packed MMs (s = 0 / 1): contraction
            # over j in gt3 rows 64s..64s+63, out partitions 64s + (2q + r).
            by_chunk = [[] for _ in range(n_chunks)]
            for i, col0, ncols in seg_plan:
                by_chunk[col0 // 512].append((i, col0, ncols))
            for q in range(n_chunks):
                sc_chunk = pssc.tile([128, 512], F32, tag="sc")
                for i, col0, ncols in by_chunk[q]:
                    c0 = col0 % 512
                    for s in range(2):
                        nc.tensor.matmul(
                            out=sc_chunk[64 * s : 64 * s + 64, c0 : c0 + ncols],
                            lhsT=gt3[64 * s : 64 * s + 64, :, :, i],
                            rhs=onehot_sb[64 * s : 64 * s + 64, col0 : col0 + ncols],
                            start=True,
                            stop=True,
                            tile_position=(64 * s, 64 * s),
                        )
                # fused: out = score * invrms2 + promo (sorted order)
                n = min(512, V - q * 512)
                cols = slice(q * 512, q * 512 + n)
                nc.vector.scalar_tensor_tensor(
                    out=final_sb[:, cols],
                    in0=sc_chunk[:, 0:n],
                    scalar=inv_sb[:, 0:1],
                    in1=promo_sb[:, cols],
                    op0=mybir.AluOpType.mult,
                    op1=mybir.AluOpType.add,
                )
                nc.sync.dma_start(out=out[:, cols], in_=final_sb[:, cols])

    nc.compile()
    return nc


_NC_CACHE = {}


def _plan_from_indices(from_idx, to_idx):
    from_idx = np.asarray(from_idx, np.int64)
    to_idx = np.asarray(to_idx, np.int64)
    order = np.argsort(from_idx, kind="stable")
    fi_sorted = from_idx[order]
    seg_plan = []
    col = 0
    for i in range(HW):
        n = int(np.count_nonzero(fi_sorted == i))
        while n > 0:
            m = min(n, 512 - col % 512)
            seg_plan.append((i, col, m))
            col += m
            n -= m
    assert col == V
    # duplicated one-hot: rows j and j+64 both = 1[to_idx[order[s]] == j]
    onehot2 = np.zeros((128, V), np.float32)
    onehot2[to_idx[order], np.arange(V)] = 1.0
    onehot2[to_idx[order] + 64, np.arange(V)] = 1.0
    return tuple(seg_plan), order, onehot2


def _host_inputs(from_w, to_w):
    def stack_w(wmat):
        wt = np.ascontiguousarray(wmat.T).reshape(CP, 2, D)  # [cp, par, d]
        lo = np.zeros((2, CP, 128), np.float32)
        hi = np.zeros((2, CP, 128), np.float32)
        lo[:, :, 0:D] = wt.transpose(1, 0, 2)
        hi[:, :, D:128] = wt.transpose(1, 0, 2)
        bf = mybir.dt.np(BF16)
        return lo.astype(bf), hi.astype(bf)

    wf_lo, wf_hi = stack_w(np.asarray(from_w, np.float32))
    wt_lo, wt_hi = stack_w(np.asarray(to_w, np.float32))
    return wf_lo, wf_hi, wt_lo, wt_hi


def _prepare(x, from_w, to_w, promo_bias, from_idx, to_idx, promo_idx):
    """Build (nc, in_maps, postprocess) for the device run."""
    seg_plan, order, onehot2 = _plan_from_indices(from_idx, to_idx)
    if seg_plan not in _NC_CACHE:
        _NC_CACHE[seg_plan] = build_kernel(seg_plan)
    nc = _NC_CACHE[seg_plan]

    wf_lo, wf_hi, wt_lo, wt_hi = _host_inputs(from_w, to_w)
    promo_sorted = np.asarray(promo_bias, np.float32)[
        np.asarray(promo_idx, np.int64)
    ][order]
    promo_rep = np.ascontiguousarray(
        np.broadcast_to(promo_sorted[None, :], (128, V)), np.float32
    )
    # marshal x per core into [cp, b, par, hw] (c = 2*cp + par)
    xr = np.asarray(x, np.float32).reshape(N_CORES, B, CP, 2, HW)
    xr = np.ascontiguousarray(xr.transpose(0, 2, 1, 3, 4))
    shared = {
        "w_f_lo": wf_lo,
        "w_f_hi": wf_hi,
        "w_t_lo": wt_lo,
        "w_t_hi": wt_hi,
        "ident": np.eye(128, dtype=np.float32),
        "s_onehot2": onehot2.astype(mybir.dt.np(BF16)),
        "promo_rep": promo_rep,
    }
    in_maps = [dict(shared, xs=xr[c]) for c in range(N_CORES)]

    sigma = _sigma()

    def post(res):
        full = np.empty((B_TOT, V), np.float32)
        for c in range(N_CORES):
            raw = res.results[c]["out"]  # rows sigma(b), cols sorted order
            logical = raw[sigma]  # rows b, cols sorted order
            full[c * B : (c + 1) * B][:, order] = logical
        return full

    return nc, in_maps, post


def kernel(
    x,
    norm_weight,
    from_w,
    from_b,
    to_w,
    to_b,
    promo_bias,
    from_idx,
    to_idx,
    promo_idx,
):
    x = np.asarray(x, np.float32)
    norm_weight = np.asarray(norm_weight, np.float32)
    from_b = np.asarray(from_b, np.float32)
    to_b = np.asarray(to_b, np.float32)

    if (
        np.any(from_b != 0.0)
        or np.any(to_b != 0.0)
        or not np.allclose(norm_weight, 1.0)
    ):
        # General-correctness fallback; never hit for this problem's input
        # distribution (norm_weight is ones, conv biases are zeros).
        return _host_reference(
            x, norm_weight, from_w, from_b, to_w, to_b, promo_bias,
            from_idx, to_idx, promo_idx,
        )

    nc, in_maps, post = _prepare(
        x, from_w, to_w, promo_bias, from_idx, to_idx, promo_idx
    )
    res = run_bass_kernel_spmd(nc, in_maps, core_ids=list(range(N_CORES)))
    return post(res)


def _host_reference(
    x, norm_weight, from_w, from_b, to_w, to_b, promo_bias, from_idx, to_idx, promo_idx
):
    b, c, w, h = x.shape
    rms = np.sqrt(np.mean(x * x, axis=(1, 2, 3), keepdims=True) + EPS)
    xn = (x / rms) * norm_weight[None]
    f = (
        np.einsum("bchw,dc->bdhw", xn, from_w) + from_b[None, :, None, None]
    ).reshape(b, -1, w * h)
    t = (
        np.einsum("bchw,dc->bdhw", xn, to_w) + to_b[None, :, None, None]
    ).reshape(b, -1, w * h)
    score = np.einsum("bdv,bdv->bv", f[:, :, from_idx], t[:, :, to_idx])
    return (score + promo_bias[promo_idx][None, :]).astype(np.float32)
